# revision 10
# baseline (speedup 1.0000x reference)
"""Trainium2 Bass kernel for nn_AttenModel (2-layer attention GNN + question LSTM).

Distribution strategy (8 NeuronCores, SPMD single program):
  * Edges are sharded by TARGET-node range: core c owns nodes [c*12500, (c+1)*12500).
    Host sorts edges by tgt, groups them per 128-node tile, pads each tile's edge
    list to a multiple of 128 (chunk) with the per-tile chunk count taken as the
    MAX across cores so all cores share one instruction stream.
  * Per edge chunk (128 edges): two indirect-DMA gathers fetch 130-element bf16
    rows [vec(128) | score_scalar | 1.0] from the node table (mx) and relation
    table (mr). Attention coeff = exp(tanh(sx+sr+ctx.aw2)). A one-hot matrix
    S[e,t] = coeff_e * (tgt_rel_e == t) is built on the vector engine and the
    segment-sum becomes PSUM-accumulated matmuls: psum += S.T @ [msg|1], giving
    numerator (128 cols) and denominator (col 129) per 128-node tile.
  * BatchNorm statistics are global: per-feature sums are accumulated with
    ones-matmuls and AllReduced across cores. The small LSTM/question context and
    relation tables are computed redundantly on every core.
  * Between layers each core computes its shard of the next gather table
    (BN+tanh via a PE transpose, then x_bn^T @ [W|W@aw]) and an AllGather
    replicates the full table to every core.
"""
import sys

if "/opt/trn_rl_repo" not in sys.path:
    sys.path.insert(0, "/opt/trn_rl_repo")

import math
import numpy as np
import ml_dtypes

import concourse.bass as bass
import concourse.bacc as bacc
import concourse.tile as tile
from concourse import mybir
from concourse.bass import IndirectOffsetOnAxis
from concourse.masks import make_identity

F32 = mybir.dt.float32
BF16 = mybir.dt.bfloat16
I32 = mybir.dt.int32
BF = ml_dtypes.bfloat16
Alu = mybir.AluOpType
Act = mybir.ActivationFunctionType

# model constants
D = 128
R = 500
NREL = 2 * R + 1
NRELP = 1024          # padded relation-table rows
SEQ = 16
L = 2
EPS = 1e-5
P = 128


def full_dims():
    return make_dims(N=100000, E=640000, NC=8, K=32, GRP=8)


def make_dims(N, E, NC, K, GRP):
    NSH = N // NC
    NT = (NSH + P - 1) // P
    SHR = NT * P
    TT = NC * NT
    ROWS = NC * SHR
    return dict(N=N, E=E, NC=NC, NSH=NSH, NT=NT, SHR=SHR, TT=TT, ROWS=ROWS,
                K=K, GRP=GRP)


# --------------------------------------------------------------------------
# host-side preprocessing
# --------------------------------------------------------------------------

def preprocess(inputs, dims):
    N, E, NC = dims["N"], dims["E"], dims["NC"]
    NSH, NT, SHR, K = dims["NSH"], dims["NT"], dims["SHR"], dims["K"]

    f32 = lambda a: np.ascontiguousarray(np.asarray(a), dtype=np.float32)
    que = f32(inputs["que_embeds"])
    x_idx = np.asarray(inputs["x"]).astype(np.int64)
    lei = np.asarray(inputs["loc_edge_index"]).astype(np.int64)
    ea = np.asarray(inputs["edge_attr"]).astype(np.int64)
    ent = f32(inputs["ent_embeds"])
    rb = f32(inputs["rel_base"])
    layers = [{k: f32(v) for k, v in lp.items()}
              for lp in inputs["params"]["layers"]]

    xe = ent[x_idx]                               # [N, D]
    src, tgt = lei[0], lei[1]

    # xeT table: [128, TT*128] bf16, col (s*NT + t)*128 + i = padded row
    xe_pad = np.zeros((dims["ROWS"], D), np.float32)
    for s in range(NC):
        xe_pad[s * SHR: s * SHR + NSH] = xe[s * NSH: (s + 1) * NSH]
    host = {"xeT": np.ascontiguousarray(xe_pad.T, dtype=BF)}

    host["queT"] = np.ascontiguousarray(que.T)          # [128, SEQ]
    host["rbT"] = np.ascontiguousarray(rb.T)            # [128, R]

    GORD = [0, 1, 3, 2]   # torch gate order i,f,g,o -> device order i,f,o,g
    for l, lp in enumerate(layers):
        W1 = lp["Wm"][:, :D]
        W2 = lp["Wm"][:, D:]
        aw1, aw2 = lp["aw"][:D], lp["aw"][D:]
        wih = np.concatenate([lp["Wih"][j * D:(j + 1) * D].T for j in GORD], axis=1)
        whh = np.concatenate([lp["Whh"][j * D:(j + 1) * D].T for j in GORD], axis=1)
        b = lp["bih"] + lp["bhh"]
        b4 = np.stack([b[j * D:(j + 1) * D] for j in GORD], axis=1)
        host[f"wihT{l}"] = np.ascontiguousarray(wih)     # [128, 512]
        host[f"whhT{l}"] = np.ascontiguousarray(whh)     # [128, 512]
        host[f"b4_{l}"] = np.ascontiguousarray(b4)       # [128, 4]
        rhsmx = np.concatenate([W1.T, (W1.T @ aw1)[:, None]], axis=1)
        host[f"rhsmx{l}"] = np.ascontiguousarray(rhsmx, dtype=BF)   # [128,129]
        host[f"w2T{l}"] = np.ascontiguousarray(W2.T)     # [128, 128]
        host[f"aw1c{l}"] = np.ascontiguousarray(aw1[:, None])
        host[f"aw2c{l}"] = np.ascontiguousarray(aw2[:, None])
        host[f"bmc{l}"] = np.ascontiguousarray(lp["bm"][:, None])
        host[f"egc{l}"] = np.ascontiguousarray(lp["eg"][:, None])
        host[f"ebc{l}"] = np.ascontiguousarray(lp["eb"][:, None])
    host["wrT0"] = np.ascontiguousarray(layers[0]["Wr"].T)
    host["brc0"] = np.ascontiguousarray(layers[0]["br"][:, None])
    host["rgc0"] = np.ascontiguousarray(layers[0]["rg"][:, None])
    host["rbc0"] = np.ascontiguousarray(layers[0]["rb"][:, None])

    # ---- edge preprocessing: sort by tgt, per-(core,tile) groups ----
    order = np.argsort(tgt, kind="stable")
    s_src, s_tgt, s_attr = src[order], tgt[order], ea[order]

    # edge group boundaries per core/tile
    grp = [[None] * NT for _ in range(NC)]
    counts = np.ones(NT, np.int64)
    for c in range(NC):
        lo = c * NSH
        i0 = np.searchsorted(s_tgt, lo)
        i1 = np.searchsorted(s_tgt, lo + NSH)
        ct, cs, ca = s_tgt[i0:i1], s_src[i0:i1], s_attr[i0:i1]
        for t in range(NT):
            j0 = np.searchsorted(ct, lo + t * P)
            j1 = np.searchsorted(ct, min(lo + (t + 1) * P, lo + NSH))
            grp[c][t] = (cs[j0:j1], ca[j0:j1], ct[j0:j1] - (lo + t * P))
            counts[t] = max(counts[t], (j1 - j0 + P - 1) // P)

    C = int(counts.sum())
    C = ((C + K - 1) // K) * K
    counts[NT - 1] += C - int(counts.sum())
    C = int(counts.sum())

    srcm = np.zeros((NC, P, C), np.int32)
    attrm = np.zeros((NC, P, C), np.int32)
    tgtm = np.full((NC, P, C), 999.0, np.float32)
    cstart = np.concatenate([[0], np.cumsum(counts)])
    for c in range(NC):
        for t in range(NT):
            cs, ca, ctr = grp[c][t]
            n = len(cs)
            slots = int(counts[t]) * P
            rows = (cs // NSH) * SHR + (cs % NSH)           # padded row ids
            pad = slots - n
            rowsp = np.concatenate([rows, np.zeros(pad, np.int64)])
            cap = np.concatenate([ca, np.zeros(pad, np.int64)])
            ctp = np.concatenate([ctr.astype(np.float32),
                                  np.full(pad, 999.0, np.float32)])
            sl = slice(int(cstart[t]), int(cstart[t + 1]))
            # edge i -> chunk i//P, lane i%P: matrix[lane, chunk]
            srcm[c, :, sl] = rowsp.reshape(-1, P).T
            attrm[c, :, sl] = cap.reshape(-1, P).T
            tgtm[c, :, sl] = ctp.reshape(-1, P).T

    return host, (srcm, attrm, tgtm), counts, C


# --------------------------------------------------------------------------
# kernel builder (one SPMD program)
# --------------------------------------------------------------------------

def build(dims, counts, C, debug=False):
    NC, NT, SHR, TT, ROWS = (dims["NC"], dims["NT"], dims["SHR"], dims["TT"],
                             dims["ROWS"])
    K, GRP = dims["K"], dims["GRP"]
    N = dims["N"]
    RG = [list(range(NC))]

    nc = bacc.Bacc("TRN2", target_bir_lowering=False, debug=False,
                   num_devices=NC)

    # ---- dram tensors
    xeT = nc.dram_tensor("xeT", [P, TT * P], BF16, kind="ExternalInput")
    queT = nc.dram_tensor("queT", [P, SEQ], F32, kind="ExternalInput")
    rbT = nc.dram_tensor("rbT", [P, R], F32, kind="ExternalInput")
    wv = {}
    for l in range(L):
        for nm, sh, dt in [(f"wihT{l}", [P, 4 * P], F32),
                           (f"whhT{l}", [P, 4 * P], F32),
                           (f"b4_{l}", [P, 4], F32),
                           (f"rhsmx{l}", [P, D + 1], BF16),
                           (f"w2T{l}", [P, P], F32),
                           (f"aw1c{l}", [P, 1], F32),
                           (f"aw2c{l}", [P, 1], F32),
                           (f"bmc{l}", [P, 1], F32),
                           (f"egc{l}", [P, 1], F32),
                           (f"ebc{l}", [P, 1], F32)]:
            wv[nm] = nc.dram_tensor(nm, sh, dt, kind="ExternalInput")
    for nm in ["wrT0"]:
        wv[nm] = nc.dram_tensor(nm, [P, P], F32, kind="ExternalInput")
    for nm in ["brc0", "rgc0", "rbc0"]:
        wv[nm] = nc.dram_tensor(nm, [P, 1], F32, kind="ExternalInput")
    srcm_d = nc.dram_tensor("srcm", [P, C], I32, kind="ExternalInput")
    attrm_d = nc.dram_tensor("attrm", [P, C], I32, kind="ExternalInput")
    tgtm_d = nc.dram_tensor("tgtm", [P, C], F32, kind="ExternalInput")

    mx1_d = nc.dram_tensor("mx1t", [ROWS, D + 2], BF16, kind="Internal")
    mrt_d = [nc.dram_tensor(f"mrt{l}", [NRELP, D + 2], BF16, kind="Internal")
             for l in range(L)]
    ag_in = nc.dram_tensor("ag_in", [SHR, D + 2], BF16, kind="Internal")
    ag_out = nc.dram_tensor("ag_out", [ROWS, D + 2], BF16, kind="Internal",
                            addr_space="Shared")
    st_in = [nc.dram_tensor(f"stin{l}", [P, 2], F32, kind="Internal")
             for l in range(L)]
    st_out = [nc.dram_tensor(f"stout{l}", [P, 2], F32, kind="Internal",
                             addr_space="Shared") for l in range(L)]
    outv = nc.dram_tensor("outv", [P, NT], F32, kind="ExternalOutput")
    dbg = {}
    if debug:
        for nm, sh, dt in [("dbg_mx1", [ROWS, D + 2], BF16),
                           ("dbg_mrt0", [NRELP, D + 2], BF16),
                           ("dbg_mrt1", [NRELP, D + 2], BF16),
                           ("dbg_agin", [SHR, D + 2], BF16),
                           ("dbg_agout", [ROWS, D + 2], BF16),
                           ("dbg_augx", [P, dims["K"] * (D + 2)], BF16),
                           ("dbg_augr", [P, dims["K"] * (D + 2)], BF16),
                           ("dbg_coef", [P, dims["K"]], F32),
                           ("dbg_x1", [P, NT * P], BF16),
                           ("dbg_x2", [P, NT * P], BF16),
                           ("dbg_r1T", [P, NRELP], F32),
                           ("dbg_gsb", [P, 4], F32),
                           ("dbg_cols", [P, 8], F32)]:
            dbg[nm] = nc.dram_tensor(nm, sh, dt, kind="ExternalOutput")

    RW = D + 2          # table row width (mx|score|1.0 / mr|score|0)

    with tile.TileContext(nc) as tc:
        with (tc.tile_pool(name="singles", bufs=1) as sg,
              tc.tile_pool(name="gen", bufs=3) as gp,
              tc.tile_pool(name="cols", bufs=8) as cp,
              tc.tile_pool(name="aug", bufs=2) as augp,
              tc.tile_pool(name="spool", bufs=4) as spl,
              tc.tile_pool(name="psgen", bufs=2, space="PSUM") as psg,
              tc.tile_pool(name="psacc", bufs=1, space="PSUM") as psa,
              tc.tile_pool(name="psx", bufs=2, space="PSUM") as psx):

            # ---------------- static SBUF ----------------
            def load(nm, sh, dt=F32):
                t = sg.tile(sh, dt, tag=nm, name=nm)
                nc.sync.dma_start(out=t[:], in_=wv[nm].ap())
                return t

            queT_sb = sg.tile([P, SEQ], F32, tag="queT", name="queT")
            nc.sync.dma_start(out=queT_sb[:], in_=queT.ap())
            wih_sb = [load(f"wihT{l}", [P, 4 * P]) for l in range(L)]
            whh_sb = [load(f"whhT{l}", [P, 4 * P]) for l in range(L)]
            b4_sb = [load(f"b4_{l}", [P, 4]) for l in range(L)]
            rhsmx_sb = [load(f"rhsmx{l}", [P, D + 1], BF16) for l in range(L)]
            w2T_sb = [load(f"w2T{l}", [P, P]) for l in range(L)]
            aw1_sb = [load(f"aw1c{l}", [P, 1]) for l in range(L)]
            aw2_sb = [load(f"aw2c{l}", [P, 1]) for l in range(L)]
            bm_sb = [load(f"bmc{l}", [P, 1]) for l in range(L)]
            eg_sb = [load(f"egc{l}", [P, 1]) for l in range(L)]
            eb_sb = [load(f"ebc{l}", [P, 1]) for l in range(L)]
            wrT0_sb = load("wrT0", [P, P])
            brc0_sb = load("brc0", [P, 1])
            rgc0_sb = load("rgc0", [P, 1])
            rbc0_sb = load("rbc0", [P, 1])

            iota_i = sg.tile([P, P], I32, tag="iota_i", name="iota_i")
            nc.gpsimd.iota(iota_i[:], pattern=[[1, P]], base=0,
                           channel_multiplier=0)
            iota_f = sg.tile([P, P], F32, tag="iota_f", name="iota_f")
            nc.vector.tensor_copy(out=iota_f[:], in_=iota_i[:])
            ident_bf = sg.tile([P, P], BF16, tag="ident_bf", name="ident_bf")
            make_identity(nc, ident_bf[:])
            ident_f = sg.tile([P, P], F32, tag="ident_f", name="ident_f")
            make_identity(nc, ident_f[:])
            ones_bf = sg.tile([P, 1], BF16, tag="ones_bf", name="ones_bf")
            nc.vector.memset(ones_bf[:], 1.0)
            ones_f = sg.tile([P, 1], F32, tag="ones_f", name="ones_f")
            nc.vector.memset(ones_f[:], 1.0)
            ones_row = sg.tile([1, P], F32, tag="ones_row", name="ones_row")
            nc.vector.memset(ones_row[:], 1.0)

            # ---------------- LSTM (replicated) ----------------
            hs_sb = [sg.tile([P, SEQ], F32, tag=f"hs{l}", name=f"hs{l}") for l in range(L)]
            ctx_sb = [sg.tile([P, 1], F32, tag=f"ctx{l}", name=f"ctx{l}") for l in range(L)]
            cdot_sb = [sg.tile([P, 1], F32, tag=f"cdot{l}", name=f"cdot{l}") for l in range(L)]

            for l in range(L):
                xin = queT_sb if l == 0 else hs_sb[l - 1]
                gx_ps = psg.tile([P, 4 * SEQ], F32, tag="g", name="g")
                for j in range(4):
                    nc.tensor.matmul(out=gx_ps[:, j * SEQ:(j + 1) * SEQ],
                                     lhsT=wih_sb[l][:, j * P:(j + 1) * P],
                                     rhs=xin[:], start=True, stop=True)
                gxb = sg.tile([P, 4 * SEQ], F32, tag=f"gxb{l}", name=f"gxb{l}")
                for j in range(4):
                    nc.vector.tensor_scalar(
                        out=gxb[:, j * SEQ:(j + 1) * SEQ],
                        in0=gx_ps[:, j * SEQ:(j + 1) * SEQ],
                        scalar1=b4_sb[l][:, j:j + 1], scalar2=None,
                        op0=Alu.add)
                gxb_v = gxb[:].rearrange("p (j t) -> p j t", t=SEQ)
                h_prev = None
                c_prev = None
                for t in range(SEQ):
                    gates = cp.tile([P, 4], F32, tag="gates", name="gates")
                    if h_prev is None:
                        # h0 == 0: gates = gxb only
                        nc.vector.tensor_copy(out=gates[:],
                                              in_=gxb_v[:, :, t])
                    else:
                        g_ps = psg.tile([P, 4], F32, tag="g", name="g")
                        for j in range(4):
                            nc.tensor.matmul(
                                out=g_ps[:, j:j + 1],
                                lhsT=whh_sb[l][:, j * P:(j + 1) * P],
                                rhs=h_prev[:],
                                start=True, stop=True)
                        nc.vector.tensor_tensor(out=gates[:], in0=g_ps[:],
                                                in1=gxb_v[:, :, t],
                                                op=Alu.add)
                    acts = cp.tile([P, 4], F32, tag="acts", name="acts")
                    nc.scalar.activation(out=acts[:, 0:3], in_=gates[:, 0:3],
                                         func=Act.Sigmoid)
                    nc.scalar.activation(out=acts[:, 3:4], in_=gates[:, 3:4],
                                         func=Act.Tanh)
                    c_new = cp.tile([P, 1], F32, tag="cst", name="cst")
                    t2 = cp.tile([P, 1], F32, tag="t2", name="t2")
                    nc.vector.tensor_tensor(out=t2[:], in0=acts[:, 0:1],
                                            in1=acts[:, 3:4], op=Alu.mult)
                    if c_prev is None:
                        nc.vector.tensor_copy(out=c_new[:], in_=t2[:])
                    else:
                        t1 = cp.tile([P, 1], F32, tag="t1", name="t1")
                        nc.vector.tensor_tensor(out=t1[:], in0=acts[:, 1:2],
                                                in1=c_prev[:], op=Alu.mult)
                        nc.vector.tensor_tensor(out=c_new[:], in0=t1[:],
                                                in1=t2[:], op=Alu.add)
                    tc_t = cp.tile([P, 1], F32, tag="tc", name="tc")
                    nc.scalar.activation(out=tc_t[:], in_=c_new[:],
                                         func=Act.Tanh)
                    nc.vector.tensor_tensor(out=hs_sb[l][:, t:t + 1],
                                            in0=acts[:, 2:3], in1=tc_t[:],
                                            op=Alu.mult)
                    h_prev = hs_sb[l][:, t:t + 1]
                    c_prev = c_new
                nc.vector.tensor_copy(out=ctx_sb[l][:], in_=c_prev[:])
                # cdot = ctx . aw2, replicated to a [P,1] column
                cd_ps = psg.tile([1, 1], F32, tag="g", name="g")
                nc.tensor.matmul(out=cd_ps[:], lhsT=ctx_sb[l][:],
                                 rhs=aw2_sb[l][:], start=True, stop=True)
                cd_sb = cp.tile([1, 1], F32, tag="cd", name="cd")
                nc.vector.tensor_copy(out=cd_sb[:], in_=cd_ps[:])
                cdr_ps = psg.tile([P, 1], F32, tag="g", name="g")
                nc.tensor.matmul(out=cdr_ps[:], lhsT=ones_row[:],
                                 rhs=cd_sb[:], start=True, stop=True)
                nc.vector.tensor_copy(out=cdot_sb[l][:], in_=cdr_ps[:])

            negctx = sg.tile([P, 1], F32, tag="negctx", name="negctx")
            nc.vector.tensor_scalar(out=negctx[:], in0=ctx_sb[L - 1][:],
                                    scalar1=-1.0, scalar2=None, op0=Alu.mult)

            # ---------------- relation tables (replicated) ----------------
            r0T = sg.tile([P, NRELP], F32, tag="r0T", name="r0T")
            nc.vector.memset(r0T[:], 0.0)
            nc.sync.dma_start(out=r0T[:, 0:R], in_=rbT.ap())
            nc.vector.tensor_scalar(out=r0T[:, R:2 * R], in0=r0T[:, 0:R],
                                    scalar1=-1.0, scalar2=None, op0=Alu.mult)
            r1T = sg.tile([P, NRELP], F32, tag="r1T", name="r1T")

            def build_mr(l, rT):
                mrT = gp.tile([P, NRELP], F32, tag="mrT", name="mrT", bufs=1)
                for i in range(NRELP // 512):
                    ps = psg.tile([P, 512], F32, tag="g", name="g")
                    nc.tensor.matmul(out=ps[:], lhsT=w2T_sb[l][:],
                                     rhs=rT[:, i * 512:(i + 1) * 512],
                                     start=True, stop=True)
                    nc.vector.tensor_scalar(out=mrT[:, i * 512:(i + 1) * 512],
                                            in0=ps[:], scalar1=bm_sb[l][:],
                                            scalar2=None, op0=Alu.add)
                for i in range(NRELP // P):
                    tp = psg.tile([P, P], F32, tag="g", name="g")
                    nc.tensor.transpose(out=tp[:], in_=mrT[:, i * P:(i + 1) * P],
                                        identity=ident_f[:])
                    stg = gp.tile([P, RW], BF16, tag="mrstg", name="mrstg")
                    nc.any.tensor_copy(out=stg[:, 0:D], in_=tp[:])
                    sp = psg.tile([P, 1], F32, tag="g", name="g")
                    nc.tensor.matmul(out=sp[:], lhsT=mrT[:, i * P:(i + 1) * P],
                                     rhs=aw1_sb[l][:], start=True, stop=True)
                    nc.any.tensor_copy(out=stg[:, D:D + 1], in_=sp[:])
                    nc.any.memset(stg[:, D + 1:RW], 0.0)
                    nc.sync.dma_start(
                        out=mrt_d[l].ap()[i * P:(i + 1) * P, :], in_=stg[:])

            build_mr(0, r0T)
            # r1 = tanh(BN(r0 @ Wr.T + br))
            zT = gp.tile([P, NRELP], F32, tag="zT", name="zT", bufs=1)
            for i in range(NRELP // 512):
                ps = psg.tile([P, 512], F32, tag="g", name="g")
                nc.tensor.matmul(out=ps[:], lhsT=wrT0_sb[:],
                                 rhs=r0T[:, i * 512:(i + 1) * 512],
                                 start=True, stop=True)
                nc.vector.tensor_scalar(out=zT[:, i * 512:(i + 1) * 512],
                                        in0=ps[:], scalar1=brc0_sb[:],
                                        scalar2=None, op0=Alu.add)
            zsum = cp.tile([P, 1], F32, tag="zsum", name="zsum")
            nc.vector.tensor_reduce(out=zsum[:], in_=zT[:, 0:NREL],
                                    axis=mybir.AxisListType.X, op=Alu.add)
            zsq = gp.tile([P, NREL], F32, tag="zsq", name="zsq", bufs=1)
            zsqs = cp.tile([P, 1], F32, tag="zsqs", name="zsqs")
            nc.scalar.activation(out=zsq[:], in_=zT[:, 0:NREL],
                                 func=Act.Square, accum_out=zsqs[:])

            def bn_cols(sum_c, sqs_c, n, gam, bet, tagp):
                mu = cp.tile([P, 1], F32, tag=tagp + "mu", name=tagp + "mu")
                nc.vector.tensor_scalar(out=mu[:], in0=sum_c[:],
                                        scalar1=1.0 / n, scalar2=None,
                                        op0=Alu.mult)
                ms = cp.tile([P, 1], F32, tag=tagp + "ms", name=tagp + "ms")
                nc.vector.tensor_scalar(out=ms[:], in0=sqs_c[:],
                                        scalar1=1.0 / n, scalar2=None,
                                        op0=Alu.mult)
                mu2 = cp.tile([P, 1], F32, tag=tagp + "mu2", name=tagp + "mu2")
                nc.vector.tensor_tensor(out=mu2[:], in0=mu[:], in1=mu[:],
                                        op=Alu.mult)
                var = cp.tile([P, 1], F32, tag=tagp + "var", name=tagp + "var")
                nc.vector.tensor_tensor(out=var[:], in0=ms[:], in1=mu2[:],
                                        op=Alu.subtract)
                vp = cp.tile([P, 1], F32, tag=tagp + "vp", name=tagp + "vp")
                nc.vector.tensor_scalar(out=vp[:], in0=var[:], scalar1=EPS,
                                        scalar2=None, op0=Alu.add)
                rv = cp.tile([P, 1], F32, tag=tagp + "rv", name=tagp + "rv")
                nc.vector.reciprocal(out=rv[:], in_=vp[:])
                rs = cp.tile([P, 1], F32, tag=tagp + "rs", name=tagp + "rs")
                nc.scalar.activation(out=rs[:], in_=rv[:], func=Act.Sqrt)
                A = sg.tile([P, 1], F32, tag=tagp + "A", name=tagp + "A")
                nc.vector.tensor_tensor(out=A[:], in0=rs[:], in1=gam[:],
                                        op=Alu.mult)
                tmb = cp.tile([P, 1], F32, tag=tagp + "tmb", name=tagp + "tmb")
                nc.vector.tensor_tensor(out=tmb[:], in0=mu[:], in1=A[:],
                                        op=Alu.mult)
                B = sg.tile([P, 1], F32, tag=tagp + "B", name=tagp + "B")
                nc.vector.tensor_tensor(out=B[:], in0=bet[:], in1=tmb[:],
                                        op=Alu.subtract)
                return A, B

            Ar, Br = bn_cols(zsum, zsqs, NREL, rgc0_sb, rbc0_sb, "r")
            rtmp = gp.tile([P, NRELP], F32, tag="rtmp", name="rtmp", bufs=1)
            nc.vector.tensor_scalar(out=rtmp[:], in0=zT[:], scalar1=Ar[:],
                                    scalar2=Br[:], op0=Alu.mult, op1=Alu.add)
            nc.scalar.activation(out=r1T[:], in_=rtmp[:], func=Act.Tanh)
            build_mr(1, r1T)

            # ---------------- MX1 table (redundant full build) -------------
            for g in range(TT // GRP):
                xe_sb = gp.tile([P, GRP * P], BF16, tag="xe", name="xe")
                nc.sync.dma_start(out=xe_sb[:],
                                  in_=xeT.ap()[:, g * GRP * P:(g + 1) * GRP * P])
                stg = gp.tile([P, GRP * RW], BF16, tag="mxstg", name="mxstg")
                for u in range(GRP):
                    ps = psg.tile([P, D + 1], F32, tag="g", name="g")
                    nc.tensor.matmul(out=ps[:],
                                     lhsT=xe_sb[:, u * P:(u + 1) * P],
                                     rhs=rhsmx_sb[0][:], start=True, stop=True)
                    nc.any.tensor_copy(out=stg[:, u * RW:u * RW + D + 1],
                                       in_=ps[:])
                    nc.any.memset(stg[:, u * RW + D + 1:(u + 1) * RW], 1.0)
                dview = mx1_d.ap()[g * GRP * P:(g + 1) * GRP * P, :]
                dview = dview.rearrange("(u p) c -> p u c", p=P)
                nc.sync.dma_start(out=dview, in_=stg[:].rearrange(
                    "p (u c) -> p u c", c=RW))

            # ---------------- edge index matrices ----------------
            srcm_sb = sg.tile([P, C], I32, tag="srcm", name="srcm")
            nc.sync.dma_start(out=srcm_sb[:], in_=srcm_d.ap())
            attrm_sb = sg.tile([P, C], I32, tag="attrm", name="attrm")
            nc.sync.dma_start(out=attrm_sb[:], in_=attrm_d.ap())
            tgtm_sb = sg.tile([P, C], F32, tag="tgtm", name="tgtm")
            nc.sync.dma_start(out=tgtm_sb[:], in_=tgtm_d.ap())

            # chunk -> tile map
            tile_of = []
            for t in range(NT):
                tile_of += [t] * int(counts[t])
            first_of = {}
            last_of = {}
            for ci, t in enumerate(tile_of):
                if t not in first_of:
                    first_of[t] = ci
                last_of[t] = ci

            x_all = [sg.tile([P, NT * P], BF16, tag=f"xall{l}", name=f"xall{l}")
                     for l in range(L)]
            stats_ps = [None, None]

            def edge_phase(l, table_d):
                stats_ps[l] = psa.tile([P, 2], F32, tag="acc", name="acc")
                xps_cur = [None]

                def do_chunk(ci, augx_t, augr_t, coeff_t, j):
                    t = tile_of[ci]
                    if ci == first_of[t]:
                        xps_cur[0] = psx.tile([P, D + 2], F32, tag="xps", name="xps")
                    xps = xps_cur[0]
                    S_t = spl.tile([P, P], BF16, tag="S", name="S")
                    nc.vector.tensor_scalar(out=S_t[:], in0=iota_f[:],
                                            scalar1=tgtm_sb[:, ci:ci + 1],
                                            scalar2=coeff_t[:, j:j + 1],
                                            op0=Alu.is_equal, op1=Alu.mult)
                    first = ci == first_of[t]
                    nc.tensor.matmul(out=xps[:, 0:D + 2],
                                     lhsT=S_t[:],
                                     rhs=augx_t[:, j * RW:(j + 1) * RW],
                                     start=first, stop=False,
                                     skip_group_check=True)
                    nc.tensor.matmul(out=xps[:, 0:D],
                                     lhsT=S_t[:],
                                     rhs=augr_t[:, j * RW:j * RW + D],
                                     start=False, stop=(ci == last_of[t]),
                                     skip_group_check=True)
                    if ci == last_of[t]:
                        dn = cp.tile([P, 1], F32, tag="dn", name="dn")
                        nc.vector.tensor_scalar(out=dn[:],
                                                in0=xps[:, D + 1:D + 2],
                                                scalar1=1e-30, scalar2=None,
                                                op0=Alu.add)
                        rv = cp.tile([P, 1], F32, tag="rvd", name="rvd")
                        nc.vector.reciprocal(out=rv[:], in_=dn[:])
                        xsl = x_all[l][:, t * P:(t + 1) * P]
                        nc.vector.tensor_scalar(out=xsl, in0=xps[:, 0:D],
                                                scalar1=rv[:], scalar2=None,
                                                op0=Alu.mult)
                        xsq = spl.tile([P, P], BF16, tag="xsq", name="xsq")
                        nc.scalar.activation(out=xsq[:], in_=xsl,
                                             func=Act.Square)
                        nc.tensor.matmul(out=stats_ps[l][:, 0:1], lhsT=xsl,
                                         rhs=ones_bf[:], start=(t == 0),
                                         stop=(t == NT - 1),
                                         skip_group_check=True)
                        nc.tensor.matmul(out=stats_ps[l][:, 1:2], lhsT=xsq[:],
                                         rhs=ones_bf[:], start=(t == 0),
                                         stop=(t == NT - 1),
                                         skip_group_check=True)

                for sc in range(C // K):
                    augx_t = augp.tile([P, K * RW], BF16, tag="augx", name="augx")
                    augr_t = augp.tile([P, K * RW], BF16, tag="augr", name="augr")
                    augx_v = augx_t[:].rearrange("p (k c) -> p k c", c=RW)
                    augr_v = augr_t[:].rearrange("p (k c) -> p k c", c=RW)
                    for j in range(K):
                        ci = sc * K + j
                        nc.gpsimd.indirect_dma_start(
                            out=augx_v[:, j, :],
                            out_offset=None, in_=table_d.ap(),
                            in_offset=IndirectOffsetOnAxis(
                                ap=srcm_sb[:, ci:ci + 1], axis=0))
                        nc.gpsimd.indirect_dma_start(
                            out=augr_v[:, j, :],
                            out_offset=None, in_=mrt_d[l].ap(),
                            in_offset=IndirectOffsetOnAxis(
                                ap=attrm_sb[:, ci:ci + 1], axis=0))
                    tc.strict_bb_all_engine_barrier()
                    sco = spl.tile([P, K], F32, tag="sco", name="sco")
                    nc.vector.tensor_tensor(
                        out=sco[:],
                        in0=augx_t[:].rearrange("p (k c) -> p k c",
                                                c=RW)[:, :, D:D + 1],
                        in1=augr_t[:].rearrange("p (k c) -> p k c",
                                                c=RW)[:, :, D:D + 1],
                        op=Alu.add)
                    tnh = spl.tile([P, K], F32, tag="tnh", name="tnh")
                    nc.scalar.activation(out=tnh[:], in_=sco[:], func=Act.Tanh,
                                         bias=cdot_sb[l][:])
                    cof = spl.tile([P, K], F32, tag="cof", name="cof")
                    nc.scalar.activation(out=cof[:], in_=tnh[:], func=Act.Exp)
                    if debug and l == 0 and sc == 0:
                        nc.sync.dma_start(out=dbg["dbg_augx"].ap(), in_=augx_t[:])
                        nc.sync.dma_start(out=dbg["dbg_augr"].ap(), in_=augr_t[:])
                        nc.sync.dma_start(out=dbg["dbg_coef"].ap(), in_=cof[:])
                    for j in range(K):
                        do_chunk(sc * K + j, augx_t, augr_t, cof, j)

            # ---------------- layer 1 ----------------
            # gathers use dynamic offsets; make sure all table writes landed
            tc.strict_bb_all_engine_barrier()
            edge_phase(0, mx1_d)

            def allreduce_stats(l):
                ssb = cp.tile([P, 2], F32, tag="ssb", name="ssb")
                nc.any.tensor_copy(out=ssb[:], in_=stats_ps[l][:])
                nc.sync.dma_start(out=st_in[l].ap(), in_=ssb[:])
                nc.gpsimd.collective_compute(
                    "AllReduce", Alu.add, replica_groups=RG,
                    ins=[st_in[l].ap()], outs=[st_out[l].ap()])
                tc.strict_bb_all_engine_barrier()
                gsb = cp.tile([P, 2], F32, tag="gsb", name="gsb")
                nc.sync.dma_start(out=gsb[:], in_=st_out[l].ap())
                if debug and l == 0:
                    nc.sync.dma_start(out=dbg["dbg_gsb"].ap()[:, 0:2], in_=gsb[:])
                return bn_cols(gsb[:, 0:1], gsb[:, 1:2], N, eg_sb[l],
                               eb_sb[l], f"x{l}")

            A0, B0 = allreduce_stats(0)

            # ---- between layers: BN + tanh + next table + AllGather
            GRP2 = 1
            for g2 in range(2, min(GRP, NT) + 1):
                if NT % g2 == 0:
                    GRP2 = g2
            for g in range(NT // GRP2):
                stg = gp.tile([P, GRP2 * RW], BF16, tag="agstg", name="agstg")
                for u in range(GRP2):
                    t = g * GRP2 + u
                    trp = psg.tile([P, P], BF16, tag="gb", name="gb")
                    nc.tensor.transpose(out=trp[:],
                                        in_=x_all[0][:, t * P:(t + 1) * P],
                                        identity=ident_bf[:])
                    bnt = spl.tile([P, P], F32, tag="bnt", name="bnt")
                    nc.vector.tensor_scalar(out=bnt[:], in0=trp[:],
                                            scalar1=A0[:], scalar2=B0[:],
                                            op0=Alu.mult, op1=Alu.add)
                    xbn = spl.tile([P, P], BF16, tag="xbn", name="xbn")
                    nc.scalar.activation(out=xbn[:], in_=bnt[:], func=Act.Tanh)
                    aps = psg.tile([P, D + 1], F32, tag="g", name="g")
                    nc.tensor.matmul(out=aps[:], lhsT=xbn[:],
                                     rhs=rhsmx_sb[1][:], start=True, stop=True)
                    nc.any.tensor_copy(out=stg[:, u * RW:u * RW + D + 1],
                                       in_=aps[:])
                    nc.any.memset(stg[:, u * RW + D + 1:(u + 1) * RW], 1.0)
                dview = ag_in.ap()[g * GRP2 * P:(g + 1) * GRP2 * P, :]
                dview = dview.rearrange("(u p) c -> p u c", p=P)
                nc.sync.dma_start(out=dview, in_=stg[:].rearrange(
                    "p (u c) -> p u c", c=RW))

            nc.gpsimd.collective_compute(
                "AllGather", Alu.bypass, replica_groups=RG,
                ins=[ag_in.ap()], outs=[ag_out.ap()])
            tc.strict_bb_all_engine_barrier()

            # ---------------- layer 2 ----------------
            edge_phase(1, ag_out)
            A1, B1 = allreduce_stats(1)

            # ---------------- final distance + sigmoid ----------------
            d2_ps = psa.tile([P, NT], F32, tag="acc", name="acc", padded_shape=[P, max(NT, 2)])
            for t in range(NT):
                trp = psg.tile([P, P], BF16, tag="gb", name="gb")
                nc.tensor.transpose(out=trp[:],
                                    in_=x_all[1][:, t * P:(t + 1) * P],
                                    identity=ident_bf[:])
                bnt = spl.tile([P, P], F32, tag="bnt", name="bnt")
                nc.vector.tensor_scalar(out=bnt[:], in0=trp[:], scalar1=A1[:],
                                        scalar2=B1[:], op0=Alu.mult,
                                        op1=Alu.add)
                xbn2 = spl.tile([P, P], F32, tag="xbn2", name="xbn2")
                nc.scalar.activation(out=xbn2[:], in_=bnt[:], func=Act.Tanh)
                dsq = spl.tile([P, P], F32, tag="dsq", name="dsq")
                nc.scalar.activation(out=dsq[:], in_=xbn2[:], func=Act.Square,
                                     bias=negctx[:])
                nc.tensor.matmul(out=d2_ps[:, t:t + 1], lhsT=dsq[:],
                                 rhs=ones_f[:], start=True, stop=True,
                                 skip_group_check=True)
            dist = sg.tile([P, NT], F32, tag="dist", name="dist")
            nc.scalar.activation(out=dist[:], in_=d2_ps[:], func=Act.Sqrt)
            osb = sg.tile([P, NT], F32, tag="osb", name="osb")
            nc.scalar.activation(out=osb[:], in_=dist[:], func=Act.Sigmoid)
            nc.sync.dma_start(out=outv.ap(), in_=osb[:])
            if debug:
                nc.sync.dma_start(out=dbg["dbg_mx1"].ap(), in_=mx1_d.ap())
                nc.sync.dma_start(out=dbg["dbg_mrt0"].ap(), in_=mrt_d[0].ap())
                nc.sync.dma_start(out=dbg["dbg_mrt1"].ap(), in_=mrt_d[1].ap())
                nc.sync.dma_start(out=dbg["dbg_agin"].ap(), in_=ag_in.ap())
                nc.sync.dma_start(out=dbg["dbg_agout"].ap(), in_=ag_out.ap())
                nc.sync.dma_start(out=dbg["dbg_x1"].ap(), in_=x_all[0][:])
                nc.sync.dma_start(out=dbg["dbg_x2"].ap(), in_=x_all[1][:])
                nc.sync.dma_start(out=dbg["dbg_r1T"].ap(), in_=r1T[:])
                dcols = sg.tile([P, 8], F32, tag="dcols", name="dcols")
                for i_, col in enumerate([A0, B0, A1, B1, cdot_sb[0],
                                          cdot_sb[1], ctx_sb[1], negctx]):
                    nc.vector.tensor_copy(out=dcols[:, i_:i_ + 1], in_=col[:])
                nc.sync.dma_start(out=dbg["dbg_cols"].ap(), in_=dcols[:])

    nc.compile()
    return nc


# --------------------------------------------------------------------------
# entry point
# --------------------------------------------------------------------------

def make_in_maps(host, idxmats, dims):
    srcm, attrm, tgtm = idxmats
    in_maps = []
    for c in range(dims["NC"]):
        m = dict(host)
        m["srcm"] = np.ascontiguousarray(srcm[c])
        m["attrm"] = np.ascontiguousarray(attrm[c])
        m["tgtm"] = np.ascontiguousarray(tgtm[c])
        in_maps.append(m)
    return in_maps


def assemble_output(results, dims):
    N, NC, NSH = dims["N"], dims["NC"], dims["NSH"]
    out = np.empty(N, np.float32)
    for c in range(NC):
        v = np.asarray(results[c]["outv"], np.float32)   # [128, NT]
        out[c * NSH:(c + 1) * NSH] = v.T.reshape(-1)[:NSH]
    return out


def kernel(**inputs):
    from concourse import bass_utils
    dims = full_dims()
    host, idxmats, counts, C = preprocess(inputs, dims)
    nc = build(dims, counts, C)
    in_maps = make_in_maps(host, idxmats, dims)
    res = bass_utils.run_bass_kernel_spmd(
        nc, in_maps, core_ids=list(range(dims["NC"])))
    return assemble_output(res.results, dims)
